# revision 3
# baseline (speedup 1.0000x reference)
"""Trainium2 Bass kernel for BiLSTM text classifier (nn_BiLSTM_73753178407543).

Reference computation (Keras-style, training-mode BN):
    mask = ids != 0
    x = embed[ids]                       # [B=128, T=1024, E=128]
    x = BN(x, axes=(0,1))                # folded into LSTM input weights
    h_f = LSTM(x, mask)      (forward)   # final hidden state [B, 128]
    h_b = LSTM(rev x, rev m) (backward)
    h = BN(concat(h_f, h_b), axes=(0,))  # folded into scale/offset
    out = softmax(h @ Wd + bd)           # [B, 10]

Strategy: data-parallel over batch, 16 examples per core on 8 cores, both
LSTM directions processed together on every core.  All on-chip tensors are
feature-major (feature dim on partitions, batch on the free dim).  The
embedding table is converted to bf16 on the host; all matmul operands are
bf16 (fp32 PSUM accumulation).

Scan design (the latency-critical part):
  - PSUM chunk bank [128, 512] holds CH=4 steps laid out
    col = j*128 + g*32 + e*16 + b  (g in [i,f,o,cc], e = dir, b = example),
    with the backward direction's steps stored at mirrored slots.
  - Per step: 8 recurrent matmuls (cc first), then tanh(cc) on the scalar
    engine (overlaps remaining matmuls), sigmoid(i,f), sigmoid(o) (off the
    critical path), then on DVE: u = [si,sf]*[tcc,c]; c' = u0+u1;
    tanh(c') on scalar; h = so*th on DVE.
  - Input projections x@W' + b' for the next chunk are issued on the PE
    during the elementwise tail of the previous step (in-order PE hides
    them in the h-dependency wait).
"""

import sys

sys.path.insert(0, "/opt/trn_rl_repo")

import numpy as np
import ml_dtypes

from concourse import bacc, mybir, tile
from concourse.bass import IndirectOffsetOnAxis
from concourse.bass_utils import run_bass_kernel_spmd
from concourse.masks import make_identity

F32 = mybir.dt.float32
BF16 = mybir.dt.bfloat16
I32 = mybir.dt.int32
AF = mybir.ActivationFunctionType
OP = mybir.AluOpType
AX = mybir.AxisListType

# Problem dims
B, T, E, H, ODIM, VOCAB = 128, 1024, 128, 128, 10, 100000
G4 = 4 * H  # 512
NCORES = 8
BL = B // NCORES  # 16 examples per core
NTOK = BL * T  # 16384 tokens per core
NBLK = NTOK // 128  # 128 gather blocks of 128 tokens
BN_EPS = 1e-3

# Kernel config
CH = 4  # LSTM steps per PSUM bank (4 steps * 4 gates * 2 dirs * 16 = 512)
GATHER_W = 4  # 128-row blocks per gather tile
TWO = 2 * BL  # 32: both dirs side by side

TRACE = False
TRACE_DIR = None
LAST_RESULT = {}


def build_program(mask_sched):
    """Build the SPMD Bass program.  mask_sched: list of (dir, step) pairs
    (identical on every core) needing masked-carry fixups; per-core mask
    data arrives via the 'mfix' input tensor."""
    nc = bacc.Bacc("TRN2", target_bir_lowering=False, debug=False,
                   num_devices=NCORES)

    NFIX = len(mask_sched)

    # ---- I/O ----
    ids_d = nc.dram_tensor("ids", [128, NBLK], I32, kind="ExternalInput")
    emb_d = nc.dram_tensor("emb", [VOCAB, E], BF16, kind="ExternalInput")
    Wf_d = nc.dram_tensor("Wf", [E, G4], F32, kind="ExternalInput")
    Wb_d = nc.dram_tensor("Wb", [E, G4], F32, kind="ExternalInput")
    Uf_d = nc.dram_tensor("Uf", [H, G4], F32, kind="ExternalInput")
    Ub_d = nc.dram_tensor("Ub", [H, G4], F32, kind="ExternalInput")
    bf_d = nc.dram_tensor("bf", [1, G4], F32, kind="ExternalInput")
    bb_d = nc.dram_tensor("bb", [1, G4], F32, kind="ExternalInput")
    g1_d = nc.dram_tensor("g1", [E, 1], F32, kind="ExternalInput")
    be1_d = nc.dram_tensor("be1", [E, 1], F32, kind="ExternalInput")
    g2_d = nc.dram_tensor("g2", [H, 2], F32, kind="ExternalInput")
    be2_d = nc.dram_tensor("be2", [H, 2], F32, kind="ExternalInput")
    Wd0_d = nc.dram_tensor("Wd0", [H, ODIM], F32, kind="ExternalInput")
    Wd1_d = nc.dram_tensor("Wd1", [H, ODIM], F32, kind="ExternalInput")
    bd_d = nc.dram_tensor("bd", [BL, ODIM], F32, kind="ExternalInput")
    gind_d = nc.dram_tensor("gind", [4, G4], BF16, kind="ExternalInput")
    if NFIX:
        mfix_d = nc.dram_tensor("mfix", [NFIX * 128, BL], mybir.dt.uint8,
                                kind="ExternalInput")
    out_d = nc.dram_tensor("out", [BL, ODIM], F32, kind="ExternalOutput")

    with tile.TileContext(nc) as tc:
        with (
            tc.tile_pool(name="const", bufs=1) as cp,
            tc.tile_pool(name="xt", bufs=1) as xp,
            tc.tile_pool(name="state", bufs=1) as sp,
            tc.tile_pool(name="step", bufs=2) as stp,
            tc.tile_pool(name="dram", bufs=1, space="DRAM") as dp,
        ):
            # ---- persistent SBUF tensors ----
            ids_sb = cp.tile([128, NBLK], I32)
            ident = cp.tile([128, 128], BF16)
            ones = cp.tile([128, 1], BF16)
            x_T = xp.tile([E, NTOK], BF16)  # embedded tokens, transposed
            w_sb = [cp.tile([E, G4], F32, tag=f"w{d}", name=f"w{d}") for d in range(2)]
            u_sb = [cp.tile([H, G4], F32, tag=f"u{d}", name=f"u{d}") for d in range(2)]
            b_sb = [cp.tile([1, G4], F32, tag=f"b{d}", name=f"b{d}") for d in range(2)]
            wq = [cp.tile([E, G4], BF16, tag=f"wq{d}", name=f"wq{d}") for d in range(2)]
            uq = [cp.tile([H, G4], BF16, tag=f"uq{d}", name=f"uq{d}") for d in range(2)]
            Bp = cp.tile([4, 2 * G4], F32)     # [g, e*512+k] both dirs' bias
            Bpq = cp.tile([4, 2 * G4], BF16)
            Gind = cp.tile([4, 2 * G4], BF16)  # indicator for the bias matmul
            wdq = [cp.tile([H, ODIM], BF16, tag=f"wdq{d}", name=f"wdq{d}") for d in range(2)]
            wd_sb = [cp.tile([H, ODIM], F32, tag=f"wd{d}", name=f"wd{d}") for d in range(2)]
            bd_sb = cp.tile([BL, ODIM], F32)
            g2_sb = cp.tile([H, 2], F32)
            be2_sb = cp.tile([H, 2], F32)
            if NFIX:
                mfix_sb = cp.tile([128, NFIX * BL], mybir.dt.uint8)

            # LSTM state: h (bf16, matmul operand), v = [tanh(cc) | c]
            h_t = sp.tile([H, TWO], BF16)   # cols 0:16 fwd, 16:32 bwd
            v_t = sp.tile([H, 2 * TWO], F32)  # [tcc(32) | c(32)]
            s_t = sp.tile([H, 3 * TWO], F32)  # [si(32) | sf(32) | so(32)]
            u_t = sp.tile([H, 2 * TWO], F32)  # products [si*tcc | sf*c]
            th_t = sp.tile([H, TWO], F32)
            # BN1 statistic tiles
            a1 = sp.tile([E, 1], F32)
            cvec = sp.tile([E, 1], F32)
            stat = sp.tile([E, 8], F32)  # scratch columns
            sq_acc = sp.tile([E, NBLK // GATHER_W], F32)
            s1 = sp.tile([1, GATHER_W * E], F32)

            nc.sync.dma_start(ids_sb[:], ids_d[:, :])
            make_identity(nc, ident[:])
            nc.vector.memset(ones[:], 1.0)
            for d, (wd_, ud_, bd_) in enumerate([(Wf_d, Uf_d, bf_d),
                                                 (Wb_d, Ub_d, bb_d)]):
                nc.sync.dma_start(w_sb[d][:], wd_[:, :])
                nc.sync.dma_start(u_sb[d][:], ud_[:, :])
                nc.sync.dma_start(b_sb[d][:], bd_[:, :])
            nc.sync.dma_start(wd_sb[0][:], Wd0_d[:, :])
            nc.sync.dma_start(wd_sb[1][:], Wd1_d[:, :])
            nc.sync.dma_start(bd_sb[:], bd_d[:, :])
            nc.sync.dma_start(g2_sb[:], g2_d[:, :])
            nc.sync.dma_start(be2_sb[:], be2_d[:, :])
            nc.sync.dma_start(Gind[:, 0:G4], gind_d[:, :])
            nc.sync.dma_start(Gind[:, G4:2 * G4], gind_d[:, :])
            if NFIX:
                for r in range(NFIX):
                    nc.sync.dma_start(
                        mfix_sb[:, r * BL:(r + 1) * BL],
                        mfix_d[r * 128:(r + 1) * 128, :])
            nc.vector.memset(h_t[:], 0.0)
            nc.vector.memset(v_t[:], 0.0)

            # ---- phase 1: gather + transpose + BN1 stats ----
            with (
                tc.tile_pool(name="nat", bufs=3) as natp,
                tc.tile_pool(name="pst", bufs=3, space="PSUM") as pstp,
                tc.tile_pool(name="pssum", bufs=1, space="PSUM") as pssp,
                tc.tile_pool(name="psprep", bufs=1, space="PSUM") as pprep,
            ):
                ps_sum = pssp.tile([1, GATHER_W * E], F32, space="PSUM")
                ngather = NBLK // GATHER_W
                for gi in range(ngather):
                    xnat = natp.tile([128, GATHER_W * E], BF16, tag="xnat")
                    for c4 in range(GATHER_W):
                        # HW indirect DMA: one index per partition, one
                        # embedding row into that partition's free extent
                        nc.gpsimd.indirect_dma_start(
                            out=xnat[:, c4 * E:(c4 + 1) * E],
                            out_offset=None,
                            in_=emb_d[:, :],
                            in_offset=IndirectOffsetOnAxis(
                                ap=ids_sb[:, gi * GATHER_W + c4:
                                          gi * GATHER_W + c4 + 1],
                                axis=0),
                        )
                    # per-channel sum over this tile's tokens (accumulated)
                    nc.tensor.matmul(
                        ps_sum[:, :GATHER_W * E], ones[:], xnat[:],
                        start=(gi == 0), stop=(gi == ngather - 1),
                        skip_group_check=True)
                    for c4 in range(GATHER_W):
                        blk = gi * GATHER_W + c4
                        pt = pstp.tile([128, 128], F32, space="PSUM",
                                       tag="pt")
                        nc.tensor.transpose(
                            pt[:], xnat[:, c4 * 128:(c4 + 1) * 128],
                            ident[:])
                        dst = x_T[:, blk * 128:(blk + 1) * 128]
                        nc.vector.tensor_copy(dst, pt[:])
                    # per-channel sum of squares of this tile's tokens
                    nc.scalar.activation(
                        s_t[:, 0:TWO].rearrange("p r -> p 1 r"),
                        x_T[:, gi * 512:(gi + 1) * 512].rearrange(
                            "p (c r) -> p c r", r=TWO),
                        AF.Square, accum_out=sq_acc[:, gi:gi + 1])

                nc.vector.tensor_reduce(stat[:, 0:1], sq_acc[:], axis=AX.X,
                                        op=OP.add)
                # collapse [1, 4*128] token-block sums -> [1, 128]
                s1g = s1[:].rearrange("p (c e) -> p c e", c=GATHER_W)
                nc.vector.tensor_copy(s1[:], ps_sum[:])
                nc.vector.tensor_tensor(s1g[:, 0], s1g[:, 0], s1g[:, 1],
                                        op=OP.add)
                nc.vector.tensor_tensor(s1g[:, 2], s1g[:, 2], s1g[:, 3],
                                        op=OP.add)
                nc.vector.tensor_tensor(s1g[:, 0], s1g[:, 0], s1g[:, 2],
                                        op=OP.add)

                # cross-core AllReduce of [sum, sumsq]
                cc_in = dp.tile([2, E], F32)
                cc_out = dp.tile([2, E], F32)
                nc.sync.dma_start(cc_in[0:1, :], s1[0:1, 0:E])
                nc.sync.dma_start(cc_in[1:2, :], stat[:, 0:1])
                nc.gpsimd.collective_compute(
                    "AllReduce", OP.add,
                    replica_groups=[list(range(NCORES))],
                    ins=[cc_in.opt()], outs=[cc_out.opt()])
                sumT = stat[:, 1:2]
                sqT = stat[:, 2:3]
                nc.sync.dma_start(sumT, cc_out[0:1, :])
                nc.sync.dma_start(sqT, cc_out[1:2, :])

                # BN1 fold:  a1 = g1 / sqrt(var+eps);  cvec = be1 - a1*mean
                ninv = 1.0 / (B * T)
                m1 = stat[:, 3:4]
                v1 = stat[:, 4:5]
                g1_sb = stat[:, 5:6]
                be1_sb = stat[:, 6:7]
                nc.sync.dma_start(g1_sb, g1_d[:, :])
                nc.sync.dma_start(be1_sb, be1_d[:, :])
                nc.vector.tensor_scalar(m1, sumT, ninv, None, op0=OP.mult)
                nc.vector.tensor_scalar(v1, sqT, ninv, None, op0=OP.mult)
                nc.vector.tensor_tensor(stat[:, 7:8], m1, m1, op=OP.mult)
                nc.vector.tensor_tensor(v1, v1, stat[:, 7:8], op=OP.subtract)
                nc.vector.tensor_scalar(v1, v1, BN_EPS, None, op0=OP.add)
                nc.scalar.activation(v1, v1, AF.Sqrt)
                nc.vector.reciprocal(v1, v1)
                nc.vector.tensor_tensor(a1[:], g1_sb, v1, op=OP.mult)
                nc.vector.tensor_tensor(stat[:, 7:8], a1[:], m1, op=OP.mult)
                nc.vector.tensor_tensor(cvec[:], be1_sb, stat[:, 7:8],
                                        op=OP.subtract)

                # weight folding per direction (gates pre-permuted on host
                # to [i, f, o, cc])
                for d in range(2):
                    psb = pprep.tile([1, G4], F32, space="PSUM", tag="psb")
                    nc.tensor.matmul(psb[:], cvec[:], w_sb[d][:],
                                     start=True, stop=True,
                                     skip_group_check=True)
                    nc.vector.tensor_tensor(b_sb[d][:], b_sb[d][:], psb[:],
                                            op=OP.add)
                    # W' = a1 * W  (per-partition scale)
                    nc.vector.tensor_scalar(w_sb[d][:], w_sb[d][:],
                                            a1[:, 0:1], None, op0=OP.mult)
                    for g in range(4):
                        nc.sync.dma_start(Bp[g:g + 1, d * G4 + g * 128:
                                             d * G4 + (g + 1) * 128],
                                          b_sb[d][0:1, g * 128:(g + 1) * 128])
                    nc.vector.tensor_copy(wq[d][:], w_sb[d][:])
                    nc.vector.tensor_copy(uq[d][:], u_sb[d][:])
                    nc.vector.tensor_copy(wdq[d][:], wd_sb[d][:])
                nc.vector.tensor_copy(Bpq[:], Bp[:])

            # ---- phase 2: the bidirectional scan ----
            fix_map = {}
            for r, (fd, fs) in enumerate(mask_sched):
                fix_map[(fd, fs)] = r

            NCHUNK = T // CH
            # per-step gate-region AP inside a bank: (g, e, b) with the bwd
            # direction at mirrored slot jm = CH-1-j
            with (
                tc.tile_pool(name="ps_scan", bufs=2, space="PSUM") as pp,
                tc.tile_pool(name="pso", bufs=1, space="PSUM") as po,
            ):
                for ck in range(NCHUNK):
                    pst = pp.tile([128, 512], F32, tag="bank", name="bank")
                    t_f = ck * CH
                    t_b = T - CH - ck * CH
                    toks = [x_T[:, t_f * BL:(t_f + CH) * BL],
                            x_T[:, t_b * BL:(t_b + CH) * BL]]
                    bank4 = pst[:].rearrange("p (j G) -> p j G", j=CH)
                    first = True
                    for e in range(2):
                        for g in range(4):
                            lo = g * 32 + e * 16
                            nc.tensor.matmul(
                                bank4[:, :, lo:lo + 16],
                                wq[e][:, g * 128:(g + 1) * 128], toks[e],
                                start=first, stop=False,
                                skip_group_check=True)
                            first = False
                    # bias add via rank-4 indicator matmul (both dirs share
                    # the gind pattern; Bpq columns select the direction)
                    nc.tensor.matmul(pst[:], Bpq[:, 0:G4], Gind[:, 0:G4],
                                     start=False, stop=False,
                                     skip_group_check=True)

                    for j in range(CH):
                        s = ck * CH + j
                        jm = [j, CH - 1 - j]   # bank slot per direction
                        # recurrent matmuls: cc, i, f then o last
                        for g in (3, 0, 1, 2):
                            for e in range(2):
                                lo = jm[e] * 128 + g * 32 + e * 16
                                nc.tensor.matmul(
                                    pst[:, lo:lo + 16],
                                    uq[e][:, g * 128:(g + 1) * 128],
                                    h_t[:, e * BL:(e + 1) * BL],
                                    start=False, stop=True,
                                    skip_group_check=True)

                        estride = (jm[1] - jm[0]) * 128 + 16

                        def gsrc(g0, ng):
                            # [p, ng, 2, 16] AP over the two dirs' regions
                            base = pst[:, jm[0] * 128 + g0 * 32:
                                       jm[0] * 128 + g0 * 32 + (ng - 1) * 32
                                       + estride + 16]
                            return base.rearrange_strided(
                                "p (g e b) -> p g e b", g=ng, e=2, b=16,
                                strides=(32, estride, 1),
                            ) if hasattr(base, 'rearrange_strided') else None

                        # tanh(cc) -> v[:, 0:32]; issued first so it runs
                        # during the remaining recurrent matmuls
                        cc_f = pst[:, jm[0] * 128 + 96:jm[0] * 128 + 112]
                        cc_b = pst[:, jm[1] * 128 + 112:jm[1] * 128 + 128]
                        nc.scalar.activation(v_t[:, 0:BL], cc_f, AF.Tanh)
                        nc.scalar.activation(v_t[:, BL:TWO], cc_b, AF.Tanh)
                        # sigmoid(i,f) -> s_t[:, 0:64]
                        if_f = pst[:, jm[0] * 128:jm[0] * 128 + 64].rearrange(
                            "p (g b) -> p g b", g=2)
                        if_b = pst[:, jm[1] * 128 + 16:
                                   jm[1] * 128 + 80].rearrange(
                            "p (g b) -> p g b", g=2)
                        st4 = s_t[:].rearrange("p (g e b) -> p g e b",
                                               g=3, e=2)
                        nc.scalar.activation(st4[:, 0:2, 0], if_f, AF.Sigmoid)
                        nc.scalar.activation(st4[:, 0:2, 1], if_b, AF.Sigmoid)
                        # sigmoid(o) -> s_t[:, 64:96] (needed only by the
                        # h update; runs during the DVE window)
                        o_f = pst[:, jm[0] * 128 + 64:jm[0] * 128 + 80]
                        o_b = pst[:, jm[1] * 128 + 80:jm[1] * 128 + 96]
                        nc.scalar.activation(s_t[:, 2 * TWO:2 * TWO + BL],
                                             o_f, AF.Sigmoid)
                        nc.scalar.activation(s_t[:, 2 * TWO + BL:3 * TWO],
                                             o_b, AF.Sigmoid)

                        fixes = [(d, fix_map[(d, s)]) for d in range(2)
                                 if (d, s) in fix_map]
                        saves = {}
                        for d, r in fixes:
                            csave = stp.tile([128, BL], F32, tag="csave")
                            hsave = stp.tile([128, BL], BF16, tag="hsave")
                            dc = slice(TWO + d * BL, TWO + (d + 1) * BL)
                            nc.vector.tensor_copy(csave[:], v_t[:, dc])
                            nc.vector.tensor_copy(
                                hsave[:], h_t[:, d * BL:(d + 1) * BL])
                            saves[d] = (csave, hsave, r)

                        # u = [si, sf] * [tcc, c]
                        nc.vector.tensor_tensor(u_t[:], s_t[:, 0:2 * TWO],
                                                v_t[:], op=OP.mult)
                        # c' = si*tcc + sf*c  -> v[:, 32:64]
                        nc.vector.tensor_tensor(v_t[:, TWO:2 * TWO],
                                                u_t[:, 0:TWO],
                                                u_t[:, TWO:2 * TWO],
                                                op=OP.add)
                        for d, (csave, hsave, r) in saves.items():
                            dc = slice(TWO + d * BL, TWO + (d + 1) * BL)
                            nc.vector.copy_predicated(
                                v_t[:, dc],
                                mfix_sb[:, r * BL:(r + 1) * BL], csave[:])
                        # th = tanh(c')
                        nc.scalar.activation(th_t[:], v_t[:, TWO:2 * TWO],
                                             AF.Tanh)
                        # h = so * th
                        nc.vector.tensor_tensor(h_t[:],
                                                s_t[:, 2 * TWO:3 * TWO],
                                                th_t[:], op=OP.mult)
                        for d, (csave, hsave, r) in saves.items():
                            nc.vector.copy_predicated(
                                h_t[:, d * BL:(d + 1) * BL],
                                mfix_sb[:, r * BL:(r + 1) * BL], hsave[:])

                # ---- phase 3: BN2 fold + dense + softmax ----
                st2 = sp.tile([H, 12], F32, tag="st2")
                scr2 = sp.tile([H, BL], F32, tag="scr2")
                for d in range(2):
                    hd = h_t[:, d * BL:(d + 1) * BL]
                    nc.vector.tensor_reduce(st2[:, 2 * d:2 * d + 1], hd,
                                            axis=AX.X, op=OP.add)
                    nc.scalar.activation(scr2[:], hd, AF.Square,
                                         accum_out=st2[:, 2 * d + 1:2 * d + 2])
                cc2_in = dp.tile([H, 4], F32, tag="cc2i")
                cc2_out = dp.tile([H, 4], F32, tag="cc2o")
                nc.sync.dma_start(cc2_in[:, :], st2[:, 0:4])
                nc.gpsimd.collective_compute(
                    "AllReduce", OP.add,
                    replica_groups=[list(range(NCORES))],
                    ins=[cc2_in.opt()], outs=[cc2_out.opt()])
                nc.sync.dma_start(st2[:, 4:8], cc2_out[:, :])

                hn = sp.tile([H, TWO], BF16, tag="hn")
                for d in range(2):
                    sm = st2[:, 4 + 2 * d:5 + 2 * d]
                    sq = st2[:, 5 + 2 * d:6 + 2 * d]
                    m2 = st2[:, 8:9]
                    v2 = st2[:, 9:10]
                    a2 = st2[:, 10:11]
                    of2 = st2[:, 11:12]
                    nc.vector.tensor_scalar(m2, sm, 1.0 / B, None,
                                            op0=OP.mult)
                    nc.vector.tensor_scalar(v2, sq, 1.0 / B, None,
                                            op0=OP.mult)
                    nc.vector.tensor_tensor(a2, m2, m2, op=OP.mult)
                    nc.vector.tensor_tensor(v2, v2, a2, op=OP.subtract)
                    nc.vector.tensor_scalar(v2, v2, BN_EPS, None, op0=OP.add)
                    nc.scalar.activation(v2, v2, AF.Sqrt)
                    nc.vector.reciprocal(v2, v2)
                    nc.vector.tensor_tensor(a2, g2_sb[:, d:d + 1], v2,
                                            op=OP.mult)
                    nc.vector.tensor_tensor(of2, a2, m2, op=OP.mult)
                    nc.vector.tensor_tensor(of2, be2_sb[:, d:d + 1], of2,
                                            op=OP.subtract)
                    nc.vector.tensor_scalar(hn[:, d * BL:(d + 1) * BL],
                                            h_t[:, d * BL:(d + 1) * BL],
                                            a2, of2, op0=OP.mult, op1=OP.add)

                ps_o = po.tile([BL, ODIM], F32, space="PSUM")
                nc.tensor.matmul(ps_o[:], hn[:, 0:BL], wdq[0][:],
                                 start=True, stop=False,
                                 skip_group_check=True)
                nc.tensor.matmul(ps_o[:], hn[:, BL:TWO], wdq[1][:],
                                 start=False, stop=True,
                                 skip_group_check=True)
                z = sp.tile([BL, ODIM], F32, tag="z")
                ez = sp.tile([BL, ODIM], F32, tag="ez")
                mx = sp.tile([BL, 2], F32, tag="mx")
                nc.vector.tensor_tensor(z[:], ps_o[:], bd_sb[:], op=OP.add)
                nc.vector.tensor_reduce(mx[:, 0:1], z[:], axis=AX.X,
                                        op=OP.max)
                nc.vector.tensor_scalar(mx[:, 1:2], mx[:, 0:1], -1.0, None,
                                        op0=OP.mult)
                nc.scalar.activation(ez[:], z[:], AF.Exp, bias=mx[:, 1:2],
                                     accum_out=mx[:, 0:1])
                nc.vector.reciprocal(mx[:, 0:1], mx[:, 0:1])
                nc.vector.tensor_scalar(z[:], ez[:], mx[:, 0:1], None,
                                        op0=OP.mult)
                nc.sync.dma_start(out_d[:, :], z[:])

    nc.finalize()
    return nc


GATE_PERM = [0, 1, 3, 2]  # keras [i, f, c, o] -> kernel [i, f, o, cc]


def _perm_gates(w):
    # w: [..., 4*H] -> permute the 4 gate blocks of the last axis
    parts = [w[..., g * H:(g + 1) * H] for g in GATE_PERM]
    return np.concatenate(parts, axis=-1)


def _prep_core_inputs(inputs, core):
    ids = np.asarray(inputs["ids"]).astype(np.int64)
    ids_c = ids[core * BL:(core + 1) * BL, :]  # [16, 1024]
    flat = ids_c.T.reshape(-1)  # token j = t*16 + b
    ids_mat = np.ascontiguousarray(
        flat.reshape(NBLK, 128).T).astype(np.int32)  # [slot p, block c]
    return ids_c, ids_mat


def kernel(**inputs):
    global LAST_RESULT
    ids = np.asarray(inputs["ids"]).astype(np.int64)

    # mask fixup schedule: union across cores of steps containing an id==0
    sched = set()
    per_core_ids = []
    for c in range(NCORES):
        ids_c, ids_mat = _prep_core_inputs(inputs, c)
        per_core_ids.append((ids_c, ids_mat))
        bs, ts = np.nonzero(ids_c == 0)
        for t in set(ts.tolist()):
            sched.add((0, int(t)))
            sched.add((1, T - 1 - int(t)))
    mask_sched = sorted(sched)
    NFIX = len(mask_sched)

    nc = build_program(mask_sched)

    emb = np.ascontiguousarray(
        np.asarray(inputs["embed_table"], dtype=np.float32)
    ).astype(ml_dtypes.bfloat16)

    # gind[q, col] = 1 iff the gate block of col is q (col = g*32+e*16+b
    # repeating every 128 within the 512-wide bank)
    gcol = (np.arange(G4) // 32) % 4
    gind = (gcol[None, :] == np.arange(4)[:, None]).astype(ml_dtypes.bfloat16)

    com = {
        "emb": emb,
        "Wf": _perm_gates(np.asarray(inputs["Wf"], np.float32)).copy(),
        "Wb": _perm_gates(np.asarray(inputs["Wb"], np.float32)).copy(),
        "Uf": _perm_gates(np.asarray(inputs["Uf"], np.float32)).copy(),
        "Ub": _perm_gates(np.asarray(inputs["Ub"], np.float32)).copy(),
        "bf": _perm_gates(np.asarray(inputs["bf"], np.float32)).reshape(1, G4).copy(),
        "bb": _perm_gates(np.asarray(inputs["bb"], np.float32)).reshape(1, G4).copy(),
        "g1": np.asarray(inputs["gamma1"], np.float32).reshape(E, 1),
        "be1": np.asarray(inputs["beta1"], np.float32).reshape(E, 1),
        "g2": np.ascontiguousarray(
            np.asarray(inputs["gamma2"], np.float32).reshape(2, H).T),
        "be2": np.ascontiguousarray(
            np.asarray(inputs["beta2"], np.float32).reshape(2, H).T),
        "Wd0": np.ascontiguousarray(
            np.asarray(inputs["Wd"], np.float32)[0:H, :]),
        "Wd1": np.ascontiguousarray(
            np.asarray(inputs["Wd"], np.float32)[H:2 * H, :]),
        "bd": np.ascontiguousarray(
            np.broadcast_to(np.asarray(inputs["bd"], np.float32), (BL, ODIM))),
        "gind": gind,
    }

    in_maps = []
    for c in range(NCORES):
        ids_c, ids_mat = per_core_ids[c]
        m = dict(com)
        m["ids"] = ids_mat
        if NFIX:
            mf = np.zeros((NFIX, 128, BL), np.uint8)
            for r, (d, s) in enumerate(mask_sched):
                t = s if d == 0 else T - 1 - s
                inv = (ids_c[:, t] == 0).astype(np.uint8)  # [16]
                mf[r, :, :] = inv[None, :]
            m["mfix"] = mf.reshape(NFIX * 128, BL)
        in_maps.append(m)

    res = run_bass_kernel_spmd(nc, in_maps, list(range(NCORES)),
                               trace=TRACE, tmpdir=TRACE_DIR)
    LAST_RESULT = {"exec_time_ns": res.exec_time_ns}
    out = np.concatenate([res.results[c]["out"] for c in range(NCORES)],
                         axis=0)
    return out.astype(np.float32)


# revision 4
# speedup vs baseline: 1.2271x; 1.2271x over previous
"""Trainium2 Bass kernel for BiLSTM text classifier (nn_BiLSTM_73753178407543).

Reference computation (Keras-style, training-mode BN):
    mask = ids != 0
    x = embed[ids]                       # [B=128, T=1024, E=128]
    x = BN(x, axes=(0,1))                # folded into LSTM input weights
    h_f = LSTM(x, mask)      (forward)   # final hidden state [B, 128]
    h_b = LSTM(rev x, rev m) (backward)
    h = BN(concat(h_f, h_b), axes=(0,))  # folded into scale/offset
    out = softmax(h @ Wd + bd)           # [B, 10]

Strategy: data-parallel over batch, 16 examples per core on 8 cores, both
LSTM directions processed together on every core.  All on-chip tensors are
feature-major (feature on partitions, batch on the free dim).  The
embedding table is converted to bf16 on the host; all matmul operands are
bf16 (fp32 PSUM accumulation).

Phase 1 builds TWO copies of the embedded sequence in SBUF: x_T in time
order and x_Tb time-reversed (via a second PE matmul against a
block-reversal permutation), so the backward scan reads ascending slices
and shares the forward code path exactly.

Scan (the latency-critical part): PSUM bank [128, 512] holds CH=4 steps,
col = j*128 + g*32 + e*16 + b  (g in [i,f,o,cc], e = direction).
Per step: 8 recurrent matmuls (cc first), tanh(cc) on the scalar engine
overlapping the remaining matmuls, sigmoid(i,f) on the chain, sigmoid(o)
off the chain, then DVE: u = [si,sf]*[tcc,c]; c' = u0+u1; tanh(c');
h = so*th.  Input projections for the next chunk issue on the PE during
the elementwise tail (in-order PE hides them in the h-dependency wait).
"""

import sys

sys.path.insert(0, "/opt/trn_rl_repo")

import numpy as np
import ml_dtypes

from concourse import bacc, mybir, tile
from concourse.bass import IndirectOffsetOnAxis
from concourse.bass_utils import run_bass_kernel_spmd

F32 = mybir.dt.float32
BF16 = mybir.dt.bfloat16
I32 = mybir.dt.int32
AF = mybir.ActivationFunctionType
OP = mybir.AluOpType
AX = mybir.AxisListType

# Problem dims
B, T, E, H, ODIM, VOCAB = 128, 1024, 128, 128, 10, 100000
G4 = 4 * H  # 512
NCORES = 8
BL = B // NCORES  # 16 examples per core
NTOK = BL * T  # 16384 tokens per core
NBLK = NTOK // 128  # 128 token blocks of 128
BN_EPS = 1e-3

# Kernel config
CH = 4  # LSTM steps per PSUM bank (4 steps * 4 gates * 2 dirs * 16 = 512)
GATHER_W = 4  # 128-row blocks per gather tile
TWO = 2 * BL  # 32: both dirs side by side

TRACE = False
TRACE_DIR = None
LAST_RESULT = {}


def build_program(mask_sched):
    """mask_sched: list of (dir, step) pairs (identical on every core)
    needing masked-carry fixups; per-core mask data arrives via 'mfix'."""
    nc = bacc.Bacc("TRN2", target_bir_lowering=False, debug=False,
                   num_devices=NCORES)

    NFIX = len(mask_sched)

    # ---- I/O ----
    ids_d = nc.dram_tensor("ids", [128, NBLK], I32, kind="ExternalInput")
    emb_d = nc.dram_tensor("emb", [VOCAB, E], BF16, kind="ExternalInput")
    Wf_d = nc.dram_tensor("Wf", [E, G4], F32, kind="ExternalInput")
    Wb_d = nc.dram_tensor("Wb", [E, G4], F32, kind="ExternalInput")
    Uf_d = nc.dram_tensor("Uf", [H, G4], F32, kind="ExternalInput")
    Ub_d = nc.dram_tensor("Ub", [H, G4], F32, kind="ExternalInput")
    bf_d = nc.dram_tensor("bf", [1, G4], F32, kind="ExternalInput")
    bb_d = nc.dram_tensor("bb", [1, G4], F32, kind="ExternalInput")
    g1_d = nc.dram_tensor("g1", [E, 1], F32, kind="ExternalInput")
    be1_d = nc.dram_tensor("be1", [E, 1], F32, kind="ExternalInput")
    g2_d = nc.dram_tensor("g2", [H, 2], F32, kind="ExternalInput")
    be2_d = nc.dram_tensor("be2", [H, 2], F32, kind="ExternalInput")
    Wd0_d = nc.dram_tensor("Wd0", [H, ODIM], F32, kind="ExternalInput")
    Wd1_d = nc.dram_tensor("Wd1", [H, ODIM], F32, kind="ExternalInput")
    bd_d = nc.dram_tensor("bd", [BL, ODIM], F32, kind="ExternalInput")
    gind_d = nc.dram_tensor("gind", [8, G4], BF16, kind="ExternalInput")
    perm_d = nc.dram_tensor("perm", [128, 2 * 128], BF16,
                            kind="ExternalInput")  # [identity | reversal]
    if NFIX:
        mfix_d = nc.dram_tensor("mfix", [NFIX * 128, BL], mybir.dt.uint8,
                                kind="ExternalInput")
    out_d = nc.dram_tensor("out", [BL, ODIM], F32, kind="ExternalOutput")

    with tile.TileContext(nc) as tc:
        with (
            tc.tile_pool(name="const", bufs=1) as cp,
            tc.tile_pool(name="xt", bufs=1) as xp,
            tc.tile_pool(name="state", bufs=1) as sp,
            tc.tile_pool(name="step", bufs=2) as stp,
            tc.tile_pool(name="dram", bufs=1, space="DRAM") as dp,
        ):
            # ---- persistent SBUF tensors ----
            ids_sb = cp.tile([128, NBLK], I32)
            perm = cp.tile([128, 2 * 128], BF16)  # [I | P_rev]
            ones = cp.tile([128, 1], BF16)
            x_T = xp.tile([E, NTOK], BF16, tag="xT", name="xT")
            x_Tb = xp.tile([E, NTOK], BF16, tag="xTb", name="xTb")
            w_sb = [cp.tile([E, G4], F32, tag=f"w{d}", name=f"w{d}") for d in range(2)]
            u_sb = [cp.tile([H, G4], F32, tag=f"u{d}", name=f"u{d}") for d in range(2)]
            b_sb = [cp.tile([1, G4], F32, tag=f"b{d}", name=f"b{d}") for d in range(2)]
            wq = [cp.tile([E, G4], BF16, tag=f"wq{d}", name=f"wq{d}") for d in range(2)]
            uq = [cp.tile([H, G4], BF16, tag=f"uq{d}", name=f"uq{d}") for d in range(2)]
            Bp = cp.tile([8, 128], F32)     # [(g,e), k] folded biases
            Bpq = cp.tile([8, 128], BF16)
            Gind = cp.tile([8, G4], BF16)   # bias indicator
            wdq = [cp.tile([H, ODIM], BF16, tag=f"wdq{d}", name=f"wdq{d}") for d in range(2)]
            wd_sb = [cp.tile([H, ODIM], F32, tag=f"wd{d}", name=f"wd{d}") for d in range(2)]
            bd_sb = cp.tile([BL, ODIM], F32)
            g2_sb = cp.tile([H, 2], F32)
            be2_sb = cp.tile([H, 2], F32)
            if NFIX:
                mfix_sb = cp.tile([128, NFIX * BL], mybir.dt.uint8)

            # LSTM state
            h_t = sp.tile([H, TWO], BF16)     # cols 0:16 fwd, 16:32 bwd
            v_t = sp.tile([H, 2 * TWO], F32)  # [tanh(cc)(32) | c(32)]
            s_t = sp.tile([H, 3 * TWO], F32)  # [si(32) | sf(32) | so(32)]
            u_t = sp.tile([H, 2 * TWO], F32)  # [si*tcc | sf*c]
            th_t = sp.tile([H, TWO], F32)
            # BN1 statistic tiles
            a1 = sp.tile([E, 1], F32)
            cvec = sp.tile([E, 1], F32)
            stat = sp.tile([E, 8], F32)
            sq_acc = sp.tile([E, NBLK // GATHER_W], F32)
            sqs = sp.tile([E, GATHER_W * 128], F32)  # Square scratch
            s1 = sp.tile([1, GATHER_W * E], F32)

            nc.sync.dma_start(ids_sb[:], ids_d[:, :])
            nc.sync.dma_start(perm[:], perm_d[:, :])
            nc.vector.memset(ones[:], 1.0)
            for d, (wd_, ud_, bd_) in enumerate([(Wf_d, Uf_d, bf_d),
                                                 (Wb_d, Ub_d, bb_d)]):
                nc.sync.dma_start(w_sb[d][:], wd_[:, :])
                nc.sync.dma_start(u_sb[d][:], ud_[:, :])
                nc.sync.dma_start(b_sb[d][:], bd_[:, :])
            nc.sync.dma_start(wd_sb[0][:], Wd0_d[:, :])
            nc.sync.dma_start(wd_sb[1][:], Wd1_d[:, :])
            nc.sync.dma_start(bd_sb[:], bd_d[:, :])
            nc.sync.dma_start(g2_sb[:], g2_d[:, :])
            nc.sync.dma_start(be2_sb[:], be2_d[:, :])
            nc.sync.dma_start(Gind[:], gind_d[:, :])
            if NFIX:
                for r in range(NFIX):
                    nc.sync.dma_start(
                        mfix_sb[:, r * BL:(r + 1) * BL],
                        mfix_d[r * 128:(r + 1) * 128, :])
            nc.vector.memset(h_t[:], 0.0)
            nc.vector.memset(v_t[:], 0.0)

            # ---- phase 1: gather + transpose (fwd & reversed) + BN1 stats
            with (
                tc.tile_pool(name="nat", bufs=3) as natp,
                tc.tile_pool(name="pst", bufs=4, space="PSUM") as pstp,
                tc.tile_pool(name="pssum", bufs=1, space="PSUM") as pssp,
                tc.tile_pool(name="psprep", bufs=1, space="PSUM") as pprep,
            ):
                ps_sum = pssp.tile([1, GATHER_W * E], F32, space="PSUM")
                ngather = NBLK // GATHER_W
                for gi in range(ngather):
                    xnat = natp.tile([128, GATHER_W * E], BF16, tag="xnat")
                    for c4 in range(GATHER_W):
                        # HW indirect DMA: one index per partition, one
                        # embedding row into that partition's free extent
                        nc.gpsimd.indirect_dma_start(
                            out=xnat[:, c4 * E:(c4 + 1) * E],
                            out_offset=None,
                            in_=emb_d[:, :],
                            in_offset=IndirectOffsetOnAxis(
                                ap=ids_sb[:, gi * GATHER_W + c4:
                                          gi * GATHER_W + c4 + 1],
                                axis=0),
                        )
                    # per-channel sum over this tile's tokens (accumulated)
                    nc.tensor.matmul(
                        ps_sum[:, :GATHER_W * E], ones[:], xnat[:],
                        start=(gi == 0), stop=(gi == ngather - 1),
                        skip_group_check=True)
                    for c4 in range(GATHER_W):
                        blk = gi * GATHER_W + c4
                        xnb = xnat[:, c4 * 128:(c4 + 1) * 128]
                        pt = pstp.tile([128, 128], F32, space="PSUM",
                                       tag="pt")
                        nc.tensor.matmul(pt[:], xnb, perm[:, 0:128],
                                         start=True, stop=True,
                                         skip_group_check=True)
                        nc.vector.tensor_copy(
                            x_T[:, blk * 128:(blk + 1) * 128], pt[:])
                        pt2 = pstp.tile([128, 128], F32, space="PSUM",
                                        tag="pt")
                        nc.tensor.matmul(pt2[:], xnb, perm[:, 128:256],
                                         start=True, stop=True,
                                         skip_group_check=True)
                        nc.scalar.copy(
                            x_Tb[:, (NBLK - 1 - blk) * 128:
                                 (NBLK - blk) * 128], pt2[:])
                    # per-channel sum of squares of this tile's tokens (DVE)
                    xs = x_T[:, gi * 512:(gi + 1) * 512]
                    nc.vector.scalar_tensor_tensor(
                        sqs[:], xs, 1.0, xs, op0=OP.mult, op1=OP.mult,
                        accum_out=sq_acc[:, gi:gi + 1])

                nc.vector.tensor_reduce(stat[:, 0:1], sq_acc[:], axis=AX.X,
                                        op=OP.add)
                # collapse [1, 4*128] token-block sums -> [1, 128]
                s1g = s1[:].rearrange("p (c e) -> p c e", c=GATHER_W)
                nc.vector.tensor_copy(s1[:], ps_sum[:])
                nc.vector.tensor_tensor(s1g[:, 0], s1g[:, 0], s1g[:, 1],
                                        op=OP.add)
                nc.vector.tensor_tensor(s1g[:, 2], s1g[:, 2], s1g[:, 3],
                                        op=OP.add)
                nc.vector.tensor_tensor(s1g[:, 0], s1g[:, 0], s1g[:, 2],
                                        op=OP.add)

                # cross-core AllReduce of [sum, sumsq]
                cc_in = dp.tile([2, E], F32)
                cc_out = dp.tile([2, E], F32)
                nc.sync.dma_start(cc_in[0:1, :], s1[0:1, 0:E])
                nc.sync.dma_start(cc_in[1:2, :], stat[:, 0:1])
                nc.gpsimd.collective_compute(
                    "AllReduce", OP.add,
                    replica_groups=[list(range(NCORES))],
                    ins=[cc_in.opt()], outs=[cc_out.opt()])
                sumT = stat[:, 1:2]
                sqT = stat[:, 2:3]
                nc.sync.dma_start(sumT, cc_out[0:1, :])
                nc.sync.dma_start(sqT, cc_out[1:2, :])

                # BN1 fold:  a1 = g1 / sqrt(var+eps);  cvec = be1 - a1*mean
                ninv = 1.0 / (B * T)
                m1 = stat[:, 3:4]
                v1 = stat[:, 4:5]
                g1_sb = stat[:, 5:6]
                be1_sb = stat[:, 6:7]
                nc.sync.dma_start(g1_sb, g1_d[:, :])
                nc.sync.dma_start(be1_sb, be1_d[:, :])
                nc.vector.tensor_scalar(m1, sumT, ninv, None, op0=OP.mult)
                nc.vector.tensor_scalar(v1, sqT, ninv, None, op0=OP.mult)
                nc.vector.tensor_tensor(stat[:, 7:8], m1, m1, op=OP.mult)
                nc.vector.tensor_tensor(v1, v1, stat[:, 7:8], op=OP.subtract)
                nc.vector.tensor_scalar(v1, v1, BN_EPS, None, op0=OP.add)
                nc.scalar.activation(v1, v1, AF.Sqrt)
                nc.vector.reciprocal(v1, v1)
                nc.vector.tensor_tensor(a1[:], g1_sb, v1, op=OP.mult)
                nc.vector.tensor_tensor(stat[:, 7:8], a1[:], m1, op=OP.mult)
                nc.vector.tensor_tensor(cvec[:], be1_sb, stat[:, 7:8],
                                        op=OP.subtract)

                # weight folding per direction (gates pre-permuted on host
                # to [i, f, o, cc])
                for d in range(2):
                    psb = pprep.tile([1, G4], F32, space="PSUM", tag="psb")
                    nc.tensor.matmul(psb[:], cvec[:], w_sb[d][:],
                                     start=True, stop=True,
                                     skip_group_check=True)
                    nc.vector.tensor_tensor(b_sb[d][:], b_sb[d][:], psb[:],
                                            op=OP.add)
                    # W' = a1 * W  (per-partition scale)
                    nc.vector.tensor_scalar(w_sb[d][:], w_sb[d][:],
                                            a1[:, 0:1], None, op0=OP.mult)
                    for g in range(4):
                        nc.sync.dma_start(Bp[2 * g + d:2 * g + d + 1, :],
                                          b_sb[d][0:1, g * 128:(g + 1) * 128])
                    nc.vector.tensor_copy(wq[d][:], w_sb[d][:])
                    nc.vector.tensor_copy(uq[d][:], u_sb[d][:])
                    nc.vector.tensor_copy(wdq[d][:], wd_sb[d][:])
                nc.vector.tensor_copy(Bpq[:], Bp[:])

            # ---- phase 2: the bidirectional scan ----
            fix_map = {}
            for r, (fd, fs) in enumerate(mask_sched):
                fix_map[(fd, fs)] = r

            NCHUNK = T // CH
            with (
                tc.tile_pool(name="ps_scan", bufs=2, space="PSUM") as pp,
                tc.tile_pool(name="pso", bufs=1, space="PSUM") as po,
            ):
                xsrc = [x_T, x_Tb]
                for ck in range(NCHUNK):
                    pst = pp.tile([128, 512], F32, space="PSUM",
                                  tag="bank", name="bank")
                    t0 = ck * CH
                    bank4 = pst[:].rearrange("p (j G) -> p j G", j=CH)
                    first = True
                    for e in range(2):
                        toks = xsrc[e][:, t0 * BL:(t0 + CH) * BL]
                        for g in range(4):
                            lo = g * 32 + e * 16
                            nc.tensor.matmul(
                                bank4[:, :, lo:lo + 16],
                                wq[e][:, g * 128:(g + 1) * 128], toks,
                                start=first, stop=False,
                                skip_group_check=True)
                            first = False
                    # bias add via rank-8 indicator matmul (both dirs)
                    nc.tensor.matmul(pst[:], Bpq[:], Gind[:],
                                     start=False, stop=False,
                                     skip_group_check=True)

                    for j in range(CH):
                        s = ck * CH + j
                        sl = pst[:, j * 128:(j + 1) * 128]
                        # recurrent matmuls: cc, i, f then o last
                        for g in (3, 0, 1, 2):
                            for e in range(2):
                                lo = g * 32 + e * 16
                                nc.tensor.matmul(
                                    sl[:, lo:lo + 16],
                                    uq[e][:, g * 128:(g + 1) * 128],
                                    h_t[:, e * BL:(e + 1) * BL],
                                    start=False, stop=True,
                                    skip_group_check=True)

                        # tanh(cc) -> v[:, 0:32]; runs during the i/f/o
                        # matmuls
                        nc.scalar.activation(v_t[:, 0:TWO], sl[:, 96:128],
                                             AF.Tanh)
                        # sigmoid(i,f) -> s_t[:, 0:64]  (the chain link)
                        nc.scalar.activation(s_t[:, 0:2 * TWO], sl[:, 0:64],
                                             AF.Sigmoid)
                        # sigmoid(o) -> s_t[:, 64:96] (off-chain)
                        nc.scalar.activation(s_t[:, 2 * TWO:3 * TWO],
                                             sl[:, 64:96], AF.Sigmoid)

                        fixes = [(d, fix_map[(d, s)]) for d in range(2)
                                 if (d, s) in fix_map]
                        saves = {}
                        for d, r in fixes:
                            csave = stp.tile([128, BL], F32, tag="csave")
                            hsave = stp.tile([128, BL], BF16, tag="hsave")
                            dc = slice(TWO + d * BL, TWO + (d + 1) * BL)
                            nc.vector.tensor_copy(csave[:], v_t[:, dc])
                            nc.vector.tensor_copy(
                                hsave[:], h_t[:, d * BL:(d + 1) * BL])
                            saves[d] = (csave, hsave, r)

                        # u = [si, sf] * [tcc, c]
                        nc.vector.tensor_tensor(u_t[:], s_t[:, 0:2 * TWO],
                                                v_t[:], op=OP.mult)
                        # c' = si*tcc + sf*c  -> v[:, 32:64]
                        nc.vector.tensor_tensor(v_t[:, TWO:2 * TWO],
                                                u_t[:, 0:TWO],
                                                u_t[:, TWO:2 * TWO],
                                                op=OP.add)
                        for d, (csave, hsave, r) in saves.items():
                            dc = slice(TWO + d * BL, TWO + (d + 1) * BL)
                            nc.vector.copy_predicated(
                                v_t[:, dc],
                                mfix_sb[:, r * BL:(r + 1) * BL], csave[:])
                        # th = tanh(c')
                        nc.scalar.activation(th_t[:], v_t[:, TWO:2 * TWO],
                                             AF.Tanh)
                        # h = so * th
                        nc.vector.tensor_tensor(h_t[:],
                                                s_t[:, 2 * TWO:3 * TWO],
                                                th_t[:], op=OP.mult)
                        for d, (csave, hsave, r) in saves.items():
                            nc.vector.copy_predicated(
                                h_t[:, d * BL:(d + 1) * BL],
                                mfix_sb[:, r * BL:(r + 1) * BL], hsave[:])

                # ---- phase 3: BN2 fold + dense + softmax ----
                st2 = sp.tile([H, 12], F32, tag="st2")
                scr2 = sp.tile([H, BL], F32, tag="scr2")
                for d in range(2):
                    hd = h_t[:, d * BL:(d + 1) * BL]
                    nc.vector.tensor_reduce(st2[:, 2 * d:2 * d + 1], hd,
                                            axis=AX.X, op=OP.add)
                    nc.scalar.activation(scr2[:], hd, AF.Square,
                                         accum_out=st2[:, 2 * d + 1:2 * d + 2])
                cc2_in = dp.tile([H, 4], F32, tag="cc2i")
                cc2_out = dp.tile([H, 4], F32, tag="cc2o")
                nc.sync.dma_start(cc2_in[:, :], st2[:, 0:4])
                nc.gpsimd.collective_compute(
                    "AllReduce", OP.add,
                    replica_groups=[list(range(NCORES))],
                    ins=[cc2_in.opt()], outs=[cc2_out.opt()])
                nc.sync.dma_start(st2[:, 4:8], cc2_out[:, :])

                hn = sp.tile([H, TWO], BF16, tag="hn")
                for d in range(2):
                    sm = st2[:, 4 + 2 * d:5 + 2 * d]
                    sq = st2[:, 5 + 2 * d:6 + 2 * d]
                    m2 = st2[:, 8:9]
                    v2 = st2[:, 9:10]
                    a2 = st2[:, 10:11]
                    of2 = st2[:, 11:12]
                    nc.vector.tensor_scalar(m2, sm, 1.0 / B, None,
                                            op0=OP.mult)
                    nc.vector.tensor_scalar(v2, sq, 1.0 / B, None,
                                            op0=OP.mult)
                    nc.vector.tensor_tensor(a2, m2, m2, op=OP.mult)
                    nc.vector.tensor_tensor(v2, v2, a2, op=OP.subtract)
                    nc.vector.tensor_scalar(v2, v2, BN_EPS, None, op0=OP.add)
                    nc.scalar.activation(v2, v2, AF.Sqrt)
                    nc.vector.reciprocal(v2, v2)
                    nc.vector.tensor_tensor(a2, g2_sb[:, d:d + 1], v2,
                                            op=OP.mult)
                    nc.vector.tensor_tensor(of2, a2, m2, op=OP.mult)
                    nc.vector.tensor_tensor(of2, be2_sb[:, d:d + 1], of2,
                                            op=OP.subtract)
                    nc.vector.tensor_scalar(hn[:, d * BL:(d + 1) * BL],
                                            h_t[:, d * BL:(d + 1) * BL],
                                            a2, of2, op0=OP.mult, op1=OP.add)

                ps_o = po.tile([BL, ODIM], F32, space="PSUM")
                nc.tensor.matmul(ps_o[:], hn[:, 0:BL], wdq[0][:],
                                 start=True, stop=False,
                                 skip_group_check=True)
                nc.tensor.matmul(ps_o[:], hn[:, BL:TWO], wdq[1][:],
                                 start=False, stop=True,
                                 skip_group_check=True)
                z = sp.tile([BL, ODIM], F32, tag="z")
                ez = sp.tile([BL, ODIM], F32, tag="ez")
                mx = sp.tile([BL, 2], F32, tag="mx")
                nc.vector.tensor_tensor(z[:], ps_o[:], bd_sb[:], op=OP.add)
                nc.vector.tensor_reduce(mx[:, 0:1], z[:], axis=AX.X,
                                        op=OP.max)
                nc.vector.tensor_scalar(mx[:, 1:2], mx[:, 0:1], -1.0, None,
                                        op0=OP.mult)
                nc.scalar.activation(ez[:], z[:], AF.Exp, bias=mx[:, 1:2],
                                     accum_out=mx[:, 0:1])
                nc.vector.reciprocal(mx[:, 0:1], mx[:, 0:1])
                nc.vector.tensor_scalar(z[:], ez[:], mx[:, 0:1], None,
                                        op0=OP.mult)
                nc.sync.dma_start(out_d[:, :], z[:])

    nc.finalize()
    return nc


GATE_PERM = [0, 1, 3, 2]  # keras [i, f, c, o] -> kernel [i, f, o, cc]


def _perm_gates(w):
    parts = [w[..., g * H:(g + 1) * H] for g in GATE_PERM]
    return np.concatenate(parts, axis=-1)


def _prep_core_inputs(inputs, core):
    ids = np.asarray(inputs["ids"]).astype(np.int64)
    ids_c = ids[core * BL:(core + 1) * BL, :]  # [16, 1024]
    flat = ids_c.T.reshape(-1)  # token j = t*16 + b
    ids_mat = np.ascontiguousarray(
        flat.reshape(NBLK, 128).T).astype(np.int32)  # [slot p, block c]
    return ids_c, ids_mat


def kernel(**inputs):
    global LAST_RESULT
    ids = np.asarray(inputs["ids"]).astype(np.int64)

    # mask fixup schedule: union across cores of steps containing an id==0
    sched = set()
    per_core_ids = []
    for c in range(NCORES):
        ids_c, ids_mat = _prep_core_inputs(inputs, c)
        per_core_ids.append((ids_c, ids_mat))
        bs, ts = np.nonzero(ids_c == 0)
        for t in set(ts.tolist()):
            sched.add((0, int(t)))
            sched.add((1, T - 1 - int(t)))
    mask_sched = sorted(sched)
    NFIX = len(mask_sched)

    nc = build_program(mask_sched)

    emb = np.ascontiguousarray(
        np.asarray(inputs["embed_table"], dtype=np.float32)
    ).astype(ml_dtypes.bfloat16)

    # bias indicator: gind[(g,e) as 2g+e, col] = 1 iff col's gate is g and
    # direction is e  (col = j*128 + g*32 + e*16 + b)
    col = np.arange(G4)
    gcol = (col // 32) % 4
    ecol = (col // 16) % 2
    q = np.arange(8)
    gind = ((gcol[None, :] == (q[:, None] // 2))
            & (ecol[None, :] == (q[:, None] % 2))).astype(ml_dtypes.bfloat16)

    # [identity | within-block time reversal] for the PE transposes
    ident = np.eye(128, dtype=ml_dtypes.bfloat16)
    c = np.arange(128)
    rev = (7 - c // 16) * 16 + c % 16
    prev_m = np.zeros((128, 128), np.float32)
    prev_m[c, rev] = 1.0
    perm = np.concatenate([ident, prev_m.astype(ml_dtypes.bfloat16)], axis=1)

    com = {
        "emb": emb,
        "Wf": _perm_gates(np.asarray(inputs["Wf"], np.float32)).copy(),
        "Wb": _perm_gates(np.asarray(inputs["Wb"], np.float32)).copy(),
        "Uf": _perm_gates(np.asarray(inputs["Uf"], np.float32)).copy(),
        "Ub": _perm_gates(np.asarray(inputs["Ub"], np.float32)).copy(),
        "bf": _perm_gates(
            np.asarray(inputs["bf"], np.float32).reshape(1, G4)).copy(),
        "bb": _perm_gates(
            np.asarray(inputs["bb"], np.float32).reshape(1, G4)).copy(),
        "g1": np.asarray(inputs["gamma1"], np.float32).reshape(E, 1),
        "be1": np.asarray(inputs["beta1"], np.float32).reshape(E, 1),
        "g2": np.ascontiguousarray(
            np.asarray(inputs["gamma2"], np.float32).reshape(2, H).T),
        "be2": np.ascontiguousarray(
            np.asarray(inputs["beta2"], np.float32).reshape(2, H).T),
        "Wd0": np.ascontiguousarray(
            np.asarray(inputs["Wd"], np.float32)[0:H, :]),
        "Wd1": np.ascontiguousarray(
            np.asarray(inputs["Wd"], np.float32)[H:2 * H, :]),
        "bd": np.ascontiguousarray(
            np.broadcast_to(np.asarray(inputs["bd"], np.float32), (BL, ODIM))),
        "gind": gind,
        "perm": perm,
    }

    in_maps = []
    for c_ in range(NCORES):
        ids_c, ids_mat = per_core_ids[c_]
        m = dict(com)
        m["ids"] = ids_mat
        if NFIX:
            mf = np.zeros((NFIX, 128, BL), np.uint8)
            for r, (d, s) in enumerate(mask_sched):
                t = s if d == 0 else T - 1 - s
                inv = (ids_c[:, t] == 0).astype(np.uint8)  # [16]
                mf[r, :, :] = inv[None, :]
            m["mfix"] = mf.reshape(NFIX * 128, BL)
        in_maps.append(m)

    res = run_bass_kernel_spmd(nc, in_maps, list(range(NCORES)),
                               trace=TRACE, tmpdir=TRACE_DIR)
    LAST_RESULT = {"exec_time_ns": res.exec_time_ns}
    out = np.concatenate([res.results[c]["out"] for c in range(NCORES)],
                         axis=0)
    return out.astype(np.float32)
